# revision 6
# baseline (speedup 1.0000x reference)
"""GAT layer (AdaptiveBreadthLayer) on 8 TRN2 NeuronCores. v3.

Strategy:
  - dst-shard: core c owns destination nodes [c*6272, (c+1)*6272) (N padded
    50000 -> 50176). Every edge lives on one core (by dst); no collectives.
  - Per-core PERMUTED node order (own dst nodes first), so each core's
    member er rows sit at table rows [t*128, (t+1)*128) with uniform code.
  - Phase 1: redundant projection of all nodes. Row = 512B:
    [248 feat bf16 | 8 feat fp8 | 4 el bf16], feat (d,h)-interleaved
    (col j = (d,h), d=j//4, h=j%4) for the per-head broadcast multiply.
    Written to two DRAM halves (rows int16-indexable for dma_gather).
    Also writes er_tab [6272, 4] bf16 for the core's own dst nodes.
  - Phase 2 walks dst-tile PAIRS. Per pair: packed dma_gathers (8-chunk /
    1024-idx instructions over the pair's lo|hi chunk streams), one-hot
    (edge -> dst-slot) build on DVE, PE transposes for the er-broadcast
    matmuls, softmax without max-subtraction, fused one-hot scatter matmul
    emitting [feat 256 | denom 4], epilogue normalize + bias + tanh +
    head-sum (the x0.25 head-mean applied on host).
"""

import sys

import numpy as np

sys.path.insert(0, "/opt/trn_rl_repo")

import ml_dtypes

import concourse.bacc as bacc
import concourse.bass as bass
import concourse.mybir as mybir
from concourse.tile import TileContext

BF16 = mybir.dt.bfloat16
FP8 = mybir.dt.float8e4
F32 = mybir.dt.float32
I32 = mybir.dt.int32
I16 = mybir.dt.int16

P = 128
H = 4
D = 64
HD = H * D  # 256
ROWB = 256  # bf16-typed row width (512 bytes)
NBF = 248  # bf16 feat cols; 248:252 hold 8 fp8 feats; 252:256 el bf16
IN_DIM = 256
NEG_SLOPE = 0.2
NW = HD + 2 * H  # 264 projection cols: [feat 256 | er 4 | el 4]
GX = HD + H  # 260 scatter cols: [feat 256 | ex 4]

N = 50000
E = 800000
NC = 8
N_PAD = 50176  # 8 * 49 * 128
NR = N_PAD // NC  # 6272 dst rows per core
TILES = NR // P  # 49 dst tiles per core
HALF = N_PAD // 2  # 25088 rows per table half (int16-indexable)
PAD_DSTLOC = 200.0  # any value >= 128: matches no dst slot
MAXC = 8  # chunks per dma_gather instruction (HW ucode cap: 1024 indices)
GT = 8  # one-hot transpose group size (PSUM bank = 8*128 bf16)
G_SZ = 2  # dst tiles per gather group


# --------------------------------------------------------------------------
# group structure (shared host/device derivation)
# --------------------------------------------------------------------------

def make_groups(clo, chi):
    groups = []
    aux_off = 0
    for g0 in range(0, TILES, G_SZ):
        ts = list(range(g0, min(g0 + G_SZ, TILES)))
        clos = [int(clo[t]) for t in ts]
        chis = [int(chi[t]) for t in ts]
        CLO, CHI = sum(clos), sum(chis)
        lo_off = [0] + list(np.cumsum(clos)[:-1])
        hi_off = [0] + list(np.cumsum(chis)[:-1])
        groups.append(
            dict(
                tiles=ts,
                clos=clos,
                chis=chis,
                CLO=CLO,
                CHI=CHI,
                C=CLO + CHI,
                lo_off=lo_off,
                hi_off=hi_off,
                aux_off=aux_off,
            )
        )
        aux_off += (CLO + CHI) * 9
    return groups, aux_off


# --------------------------------------------------------------------------
# host-side preprocessing (index structures only; no float math)
# --------------------------------------------------------------------------

def preprocess(src, dst):
    src = np.asarray(src).astype(np.int64)
    dst = np.asarray(dst).astype(np.int64)
    bf = ml_dtypes.bfloat16
    core_of = dst // NR
    percore = []
    lo_cnt = np.zeros((NC, TILES), np.int64)
    hi_cnt = np.zeros((NC, TILES), np.int64)
    for c in range(NC):
        m = core_of == c
        s, d = src[m], dst[m]
        base = c * NR
        # permuted position: own range first, then the rest in order
        pos = np.where(s >= base, np.where(s < base + NR, s - base, s), s + NR)
        is_lo = pos < HALF
        tl = (d - base) // P
        sl = (d - base) % P
        percore.append((pos, is_lo, tl, sl))
        np.add.at(lo_cnt[c], tl[is_lo], 1)
        np.add.at(hi_cnt[c], tl[~is_lo], 1)
    clo = np.maximum(1, np.ceil(lo_cnt.max(axis=0) / P)).astype(np.int64)
    chi = np.maximum(1, np.ceil(hi_cnt.max(axis=0) / P)).astype(np.int64)
    clo = [int(x) for x in clo]
    chi = [int(x) for x in chi]
    groups, sum_w = make_groups(clo, chi)

    # per-tile lookup tables
    gi_of = np.zeros(TILES, np.int64)
    lo_off_t = np.zeros(TILES, np.int64)
    hi_off_t = np.zeros(TILES, np.int64)
    for gi, g in enumerate(groups):
        for j, t in enumerate(g["tiles"]):
            gi_of[t] = gi
            lo_off_t[t] = g["lo_off"][j]
            hi_off_t[t] = g["hi_off"][j]
    g_aux = np.array([g["aux_off"] for g in groups])
    g_clo = np.array([g["CLO"] for g in groups])
    g_C = np.array([g["C"] for g in groups])
    cum_C = np.concatenate([[0], np.cumsum(g_C)])
    sumC = int(cum_C[-1])

    aux = []
    for c in range(NC):
        pos, is_lo, tl, sl = percore[c]
        idxf = np.zeros((16, sumC * 8), np.int16)
        dlf = np.full((P, sumC), PAD_DSTLOC, dtype=bf)
        for half in (True, False):
            selh = is_lo == half
            t_h = tl[selh]
            p_h = sl[selh]
            pos_h = pos[selh] - (0 if half else HALF)
            order = np.argsort(t_h, kind="stable")
            t_s, pos_s, p_s = t_h[order], pos_h[order], p_h[order]
            tile_starts = np.searchsorted(t_s, np.arange(TILES))
            q = np.arange(len(order)) - tile_starts[t_s]
            chunk_local = q // P
            slot = q % P
            gi = gi_of[t_s]
            if half:
                cpos = cum_C[gi] + lo_off_t[t_s] + chunk_local
            else:
                cpos = cum_C[gi] + g_clo[gi] + hi_off_t[t_s] + chunk_local
            idxf[slot % 16, cpos * 8 + slot // 16] = pos_s.astype(np.int16)
            dlf[slot, cpos] = p_s.astype(bf)
        auxw = np.zeros((P, sum_w), np.int16)
        for gi, g in enumerate(groups):
            a0, C = g["aux_off"], g["C"]
            blk = idxf[:, cum_C[gi] * 8 : cum_C[gi + 1] * 8]
            auxw[:, a0 : a0 + C * 8] = np.tile(blk, (8, 1))
            auxw[:, a0 + C * 8 : a0 + C * 9] = dlf[
                :, cum_C[gi] : cum_C[gi + 1]
            ].view(np.int16)
        aux.append(auxw)
    return aux, clo, chi, groups


# --------------------------------------------------------------------------
# device kernel builder
# --------------------------------------------------------------------------

def build_kernel(clo, chi, rep1=1, rep2=1, nq=2, scratch=32768):
    groups, sum_w = make_groups(clo, chi)
    cmaxg = max(g["C"] for g in groups)
    nc = bacc.Bacc(dynamic_dma_scratch_size=scratch, num_swdge_queues=nq)

    hT = nc.declare_dram_parameter("hT", [IN_DIM, N_PAD], BF16, isOutput=False)
    Wfull = nc.declare_dram_parameter("Wfull", [IN_DIM, NW], BF16, isOutput=False)
    bias_rep = nc.declare_dram_parameter("bias_rep", [P, HD], F32, isOutput=False)
    iota_big = nc.declare_dram_parameter(
        "iota_big", [P, P * cmaxg], BF16, isOutput=False
    )
    ident = nc.declare_dram_parameter("ident", [P, P], BF16, isOutput=False)
    auxw = nc.declare_dram_parameter("auxw", [P, sum_w], I16, isOutput=False)
    out = nc.declare_dram_parameter("out", [TILES * P, D], F32, isOutput=True)

    AL = mybir.AluOpType
    AF = mybir.ActivationFunctionType
    KCH = IN_DIM // P  # 2 contraction chunks
    build_kernel._gq = 0

    with TileContext(nc) as tc:
        with (
            tc.tile_pool(name="const", bufs=1) as constp,
            tc.tile_pool(name="dram", bufs=1, space="DRAM") as dramp,
        ):
            t_lo = dramp.tile([HALF, ROWB], BF16)
            t_hi = dramp.tile([HALF, ROWB], BF16)
            er_tab = dramp.tile([NR, H], BF16)

            W_sb = constp.tile([P, KCH * NW], BF16)
            bias_sb = constp.tile([P, HD], F32)
            iota_sb = constp.tile([P, P * cmaxg], BF16)
            ident_sb = constp.tile([P, P], BF16)
            for kk in range(KCH):
                nc.sync.dma_start(
                    out=W_sb[:, kk * NW : (kk + 1) * NW],
                    in_=Wfull[kk * P : (kk + 1) * P, :],
                )
            nc.sync.dma_start(out=bias_sb[:], in_=bias_rep[:, :])
            nc.sync.dma_start(out=iota_sb[:], in_=iota_big[:, :])
            nc.sync.dma_start(out=ident_sb[:], in_=ident[:, :])
            aux_all = constp.tile([P, sum_w], I16)
            nc.sync.dma_start(out=aux_all[:], in_=auxw[:, :])
            out_all = constp.tile([P, TILES, D], F32)
            er_all = constp.tile([P, TILES, H], BF16)

            # ------------------- phase 1: projection table -------------------
            OB = 512  # rows per outer block
            SUBS = OB // P  # 4
            n_ob = N_PAD // OB
            SPL = 152  # stage copy split: Act does 0:SPL, DVE does SPL:NBF
            for _r1 in range(rep1):
              with (
                tc.tile_pool(name="p1", bufs=3) as p1,
                tc.tile_pool(name="p1ps", bufs=2, space="PSUM") as p1ps,
              ):
                hT_tiles = {}

                def load_block(tt):
                    ht = p1.tile([P, KCH, OB], BF16, name="hT_t", tag="hT_t")
                    nc.sync.dma_start(
                        out=ht[:],
                        in_=hT[:, tt * OB : (tt + 1) * OB].rearrange(
                            "(k p) c -> p k c", p=P
                        ),
                    )
                    hT_tiles[tt] = ht

                load_block(0)
                load_block(1)
                for ob in range(n_ob):
                    start = ob * OB
                    hT_t = hT_tiles.pop(ob)
                    # 4 banks: sub s occupies bank s: [feat 256 | er 4 | el 4]
                    pbig = p1ps.tile([P, SUBS, 512], F32, name="pbig", tag="pbig")
                    for sub in range(SUBS):
                        for kk in range(KCH):
                            nc.tensor.matmul(
                                pbig[:, sub, 0:NW],
                                lhsT=hT_t[:, kk, sub * P : (sub + 1) * P],
                                rhs=W_sb[:, kk * NW : (kk + 1) * NW],
                                start=(kk == 0),
                                stop=(kk == KCH - 1),
                            )
                    stage = p1.tile([P, SUBS, ROWB], BF16, name="stage", tag="stage")
                    nc.scalar.copy(out=stage[:, :, 0:SPL], in_=pbig[:, :, 0:SPL])
                    nc.vector.tensor_copy(
                        out=stage[:, :, SPL:NBF], in_=pbig[:, :, SPL:NBF]
                    )
                    # 8 fp8 feats <- psum cols 248:256
                    nc.vector.tensor_copy(
                        out=stage[:, :, NBF : NBF + 4].bitcast(FP8),
                        in_=pbig[:, :, NBF : NBF + 8],
                    )
                    # el bf16 <- psum cols 260:264
                    nc.vector.tensor_copy(
                        out=stage[:, :, 252:256], in_=pbig[:, :, 260:264]
                    )
                    if start < NR:
                        nsub = SUBS if start + OB <= NR else (NR - start) // P
                        er_st = p1.tile([P, SUBS, H], BF16, name="er_st", tag="er_st")
                        nc.vector.tensor_copy(
                            out=er_st[:, 0:nsub, :], in_=pbig[:, 0:nsub, 256:260]
                        )
                        nc.sync.dma_start(
                            out=er_tab[start : start + nsub * P, :].rearrange(
                                "(s p) h -> p s h", p=P
                            ),
                            in_=er_st[:, 0:nsub, :],
                        )
                    # prefetch next-next hT block behind the stage copies
                    if ob + 2 < n_ob:
                        load_block(ob + 2)
                    tgt, r0 = (t_lo, start) if start < HALF else (t_hi, start - HALF)
                    dst_ap = tgt[r0 : r0 + OB, :].rearrange("(s p) c -> p s c", p=P)
                    nc.sync.dma_start(out=dst_ap, in_=stage[:])

            # ------------------- phase 2: edge aggregation -------------------
            for _r2 in range(rep2):
              with (
                tc.tile_pool(name="p2", bufs=3) as p2,
                tc.tile_pool(name="p2g", bufs=2) as p2g,
                tc.tile_pool(name="p2s", bufs=4) as p2s,
                tc.tile_pool(name="ssb", bufs=6) as ssb_pool,
                tc.tile_pool(name="outps", bufs=4, space="PSUM") as outps_pool,
                tc.tile_pool(name="ergps", bufs=2, space="PSUM") as ergps_pool,
                tc.tile_pool(name="sps", bufs=2, space="PSUM") as sps_pool,
              ):
                nc.sync.dma_start(
                    out=er_all[:],
                    in_=er_tab[:, :].rearrange("(t p) h -> p t h", p=P),
                )
                for g in groups:
                    C, CLO, CHI = g["C"], g["CLO"], g["CHI"]
                    a0 = g["aux_off"]
                    aux_g = aux_all[:, a0 : a0 + C * 9]
                    idx_v = aux_g[:, 0 : C * 8]
                    dl_v = aux_g[:, C * 8 : C * 9].bitcast(BF16)

                    G = p2g.tile([P, C, ROWB], BF16, name="G", tag="G")
                    for b0, width, tb in ((0, CLO, t_lo), (CLO, CHI, t_hi)):
                        done = 0
                        while done < width:
                            w = min(MAXC, width - done)
                            b = b0 + done
                            nc.gpsimd.dma_gather(
                                out_ap=G[:, b : b + w, :],
                                in_ap=tb[:, :],
                                idxs_ap=idx_v[:, b * 8 : (b + w) * 8],
                                num_idxs=w * P,
                                num_idxs_reg=w * P,
                                elem_size=ROWB,
                                queue_num=build_kernel._gq % nq,
                            )
                            build_kernel._gq += 1
                            done += w

                    # one-hot over all the group's chunks, chunk-innermost
                    # (DVE 2x): ST3[e, d, j] = (dstloc[e, j] == d)
                    ST3 = p2.tile([P, P * C], BF16, name="ST3", tag="ST")
                    nc.vector.tensor_tensor(
                        out=ST3[:].rearrange("p (d c) -> p d c", d=P),
                        in0=dl_v.rearrange("p (one c) -> p one c", one=1)
                        .to_broadcast([P, P, C]),
                        in1=iota_sb[:]
                        .rearrange("p (d c) -> p d c", d=P)[:, :, 0:C],
                        op=AL.is_equal,
                    )
                    st_j = lambda j: ST3[:].rearrange("p (d c) -> p d c", d=P)[
                        :, :, j
                    ]

                    # transposed one-hot blocks (for the erg matmuls)
                    sbs = []
                    for q0 in range(0, C, GT):
                        q1 = min(q0 + GT, C)
                        s_ps = sps_pool.tile(
                            [P, GT, P], BF16, name="s_ps", tag="s_ps"
                        )
                        for j in range(q0, q1):
                            nc.tensor.transpose(
                                out=s_ps[:, j - q0, :],
                                in_=st_j(j),
                                identity=ident_sb[:],
                            )
                        s_sb = ssb_pool.tile(
                            [P, GT, P], BF16, name="s_sb", tag="s_sb"
                        )
                        nc.scalar.copy(
                            out=s_sb[:, 0 : q1 - q0, :], in_=s_ps[:, 0 : q1 - q0, :]
                        )
                        sbs.append((q0, q1, s_sb))

                    # chunk -> tile map (group chunk order: los then his)
                    t_of = []
                    for j_t, t in enumerate(g["tiles"]):
                        t_of += [t] * g["clos"][j_t]
                    for j_t, t in enumerate(g["tiles"]):
                        t_of += [t] * g["chis"][j_t]

                    # er gathered per edge: erg[e, h] = er_all[dstloc[e], t, h]
                    erg_ps = ergps_pool.tile([P, C * H], F32, name="erg_ps")
                    for q0, q1, s_sb in sbs:
                        for j in range(q0, q1):
                            nc.tensor.matmul(
                                erg_ps[:, j * H : (j + 1) * H],
                                lhsT=s_sb[:, j - q0, :],
                                rhs=er_all[:, t_of[j], :],
                                start=True,
                                stop=True,
                            )

                    # e_val = leaky_relu(el[src] + er[dst]); ex = exp(e_val)
                    ev = p2.tile([P, C * H], F32, name="ev", tag="ev")
                    nc.vector.tensor_tensor(
                        out=ev[:].rearrange("p (c h) -> p c h", c=C),
                        in0=G[:, :, 252:256],
                        in1=erg_ps[:].rearrange("p (c h) -> p c h", c=C),
                        op=AL.add,
                    )
                    lrel = p2.tile([P, C * H], F32, name="lrel", tag="lrel")
                    nc.vector.scalar_tensor_tensor(
                        out=lrel[:],
                        in0=ev[:],
                        scalar=NEG_SLOPE,
                        in1=ev[:],
                        op0=AL.mult,
                        op1=AL.max,
                    )
                    gx = p2.tile([P, C, GX], BF16, name="gx", tag="gx")
                    nc.scalar.activation(
                        out=gx[:, :, HD:GX],
                        in_=lrel[:].rearrange("p (c h) -> p c h", c=C),
                        func=AF.Exp,
                    )
                    exv = gx[:, :, HD:GX].rearrange(
                        "p c (one h) -> p c one h", one=1
                    )
                    # 8 fp8 feats -> bf16
                    Gx8 = p2.tile([P, C, 8], BF16, name="Gx8", tag="Gx8")
                    nc.scalar.copy(
                        out=Gx8[:], in_=G[:, :, NBF : NBF + 4].bitcast(FP8)
                    )
                    nc.vector.tensor_tensor(
                        out=gx[:, :, 0:NBF].rearrange("p c (d h) -> p c d h", h=H),
                        in0=G[:, :, 0:NBF].rearrange("p c (d h) -> p c d h", h=H),
                        in1=exv.to_broadcast([P, C, NBF // H, H]),
                        op=AL.mult,
                    )
                    nc.vector.tensor_tensor(
                        out=gx[:, :, NBF:HD].rearrange("p c (d h) -> p c d h", h=H),
                        in0=Gx8[:].rearrange("p c (d h) -> p c d h", h=H),
                        in1=exv.to_broadcast([P, C, (HD - NBF) // H, H]),
                        op=AL.mult,
                    )

                    # scatter matmuls + epilogue, one dst tile at a time
                    for j_t, t in enumerate(g["tiles"]):
                        chunks = list(
                            range(g["lo_off"][j_t], g["lo_off"][j_t] + g["clos"][j_t])
                        ) + list(
                            range(
                                CLO + g["hi_off"][j_t],
                                CLO + g["hi_off"][j_t] + g["chis"][j_t],
                            )
                        )
                        out_ps = outps_pool.tile([P, GX], F32, name="out_ps")
                        for k, j in enumerate(chunks):
                            nc.tensor.matmul(
                                out_ps[:],
                                lhsT=st_j(j),
                                rhs=gx[:, j, :],
                                start=(k == 0),
                                stop=(k == len(chunks) - 1),
                            )
                        # normalize, bias, tanh, head-sum (x0.25 on host)
                        rd = p2s.tile([P, H], F32, name="rd", tag="rd")
                        nc.vector.reciprocal(out=rd[:], in_=out_ps[:, HD:GX])
                        nrm = p2.tile([P, HD], F32, name="nrm", tag="nrm")
                        nc.vector.tensor_tensor(
                            out=nrm[:].rearrange("p (d h) -> p d h", h=H),
                            in0=out_ps[:, 0:HD].rearrange("p (d h) -> p d h", h=H),
                            in1=rd[:]
                            .rearrange("p (one h) -> p one h", one=1)
                            .to_broadcast([P, D, H]),
                            op=AL.mult,
                        )
                        nb = p2.tile([P, HD], F32, name="nb", tag="nb")
                        nb_eng = nc.vector if t % 2 == 0 else nc.gpsimd
                        nb_eng.tensor_tensor(
                            out=nb[:], in0=nrm[:], in1=bias_sb[:], op=AL.add
                        )
                        th = p2.tile([P, HD], F32, name="th", tag="th")
                        nc.scalar.activation(out=th[:], in_=nb[:], func=AF.Tanh)
                        thv = th[:].rearrange("p (d h) -> p d h", h=H)
                        m1 = p2s.tile([P, D], F32, name="m1", tag="m1")
                        nc.vector.tensor_tensor(
                            out=m1[:], in0=thv[:, :, 0], in1=thv[:, :, 1], op=AL.add
                        )
                        m2 = p2s.tile([P, D], F32, name="m2", tag="m2")
                        nc.vector.tensor_tensor(
                            out=m2[:], in0=thv[:, :, 2], in1=thv[:, :, 3], op=AL.add
                        )
                        nc.gpsimd.tensor_tensor(
                            out=out_all[:, t, :], in0=m1[:], in1=m2[:], op=AL.add
                        )
                nc.sync.dma_start(
                    out=out[:, :].rearrange("(t p) d -> p t d", p=P), in_=out_all[:]
                )
    return nc


# --------------------------------------------------------------------------
# host entry
# --------------------------------------------------------------------------

# feat col j = (d, h): feat_new[:, j] = feat_old[:, (j % 4) * 64 + j // 4]
_PERM = np.array([(j % H) * D + j // H for j in range(HD)], dtype=np.int64)


def _make_static_inputs(h, W, attn_l, attn_r, bias, cmaxg):
    bf = ml_dtypes.bfloat16
    h_pad = np.zeros((N_PAD, IN_DIM), dtype=np.float32)
    h_pad[:N] = np.asarray(h, dtype=np.float32)
    W_perm = np.asarray(W, dtype=np.float32)[:, _PERM]
    ALRm = np.zeros((HD, 2 * H), dtype=np.float32)
    al = np.asarray(attn_l, dtype=np.float32)
    ar = np.asarray(attn_r, dtype=np.float32)
    for j in range(HD):
        d, hh = j // H, j % H
        ALRm[j, hh] = ar[hh, d]  # er first
        ALRm[j, H + hh] = al[hh, d]  # el last
    Wfull = np.concatenate([W_perm, W_perm @ ALRm], axis=1).astype(bf)
    bias_perm = np.asarray(bias, dtype=np.float32)[_PERM]
    bias_rep = np.tile(bias_perm.reshape(1, HD), (P, 1))
    ident = np.eye(P, dtype=np.float32).astype(bf)
    # iota_big[p, d*cmaxg + j] = d  (chunk-innermost iota for the ST3 build)
    row = np.repeat(np.arange(P, dtype=np.float32), cmaxg).reshape(1, P * cmaxg)
    iota_big = np.ascontiguousarray(np.tile(row, (P, 1)).astype(bf))
    shared = dict(
        Wfull=np.ascontiguousarray(Wfull),
        bias_rep=np.ascontiguousarray(bias_rep),
        ident=np.ascontiguousarray(ident),
        iota_big=iota_big,
    )
    hTs = []
    for c in range(NC):
        base = c * NR
        order = np.concatenate(
            [
                np.arange(base, base + NR),
                np.arange(0, base),
                np.arange(base + NR, N_PAD),
            ]
        )
        hTs.append(np.ascontiguousarray(h_pad[order].T.astype(bf)))
    return shared, hTs


def bench(nc, in_maps, n_iters=10):
    """Repeated-execution wall timing of the compiled SPMD kernel via PJRT.

    Returns (per_call_seconds_list, results_of_last_call)."""
    import time

    import jax
    from jax.sharding import Mesh, NamedSharding, PartitionSpec
    from jax.experimental.shard_map import shard_map

    from concourse import bass2jax, mybir as _mb

    bass2jax.install_neuronx_cc_hook()
    n_cores = len(in_maps)
    in_names, out_names, out_avals, zero_outs = [], [], [], []
    partition_name = nc.partition_id_tensor.name if nc.partition_id_tensor else None
    for alloc in nc.m.functions[0].allocations:
        if not isinstance(alloc, _mb.MemoryLocationSet):
            continue
        name = alloc.memorylocations[0].name
        if alloc.kind == "ExternalInput":
            if name != partition_name:
                in_names.append(name)
        elif alloc.kind == "ExternalOutput":
            out_names.append(name)
            shape = tuple(alloc.tensor_shape)
            dtype = _mb.dt.np(alloc.dtype)
            out_avals.append(jax.core.ShapedArray(shape, dtype))
            zero_outs.append(np.zeros(shape, dtype))
    n_params = len(in_names)
    all_in_names = in_names + out_names
    if partition_name is not None:
        all_in_names.append(partition_name)

    def _body(*args):
        operands = list(args)
        if partition_name is not None:
            operands.append(bass2jax.partition_id_tensor())
        outs = bass2jax._bass_exec_p.bind(
            *operands,
            out_avals=tuple(out_avals),
            in_names=tuple(all_in_names),
            out_names=tuple(out_names),
            lowering_input_output_aliases=(),
            sim_require_finite=True,
            sim_require_nnan=True,
            nc=nc,
        )
        return tuple(outs)

    devices = jax.devices()[:n_cores]
    mesh = Mesh(np.asarray(devices), ("core",))
    n_outs = len(out_names)
    sharded = jax.jit(
        shard_map(
            _body,
            mesh=mesh,
            in_specs=(PartitionSpec("core"),) * (n_params + n_outs),
            out_specs=(PartitionSpec("core"),) * n_outs,
            check_rep=False,
        ),
        keep_unused=True,
    )
    sh = NamedSharding(mesh, PartitionSpec("core"))
    concat_in = [
        jax.device_put(
            np.concatenate([np.asarray(in_maps[c][nm]) for c in range(n_cores)], 0), sh
        )
        for nm in in_names
    ]
    concat_zeros = [
        jax.device_put(np.zeros((n_cores * z.shape[0], *z.shape[1:]), z.dtype), sh)
        for z in zero_outs
    ]
    outs = sharded(*concat_in, *concat_zeros)  # warmup/compile
    jax.block_until_ready(outs)
    times = []
    for _ in range(n_iters):
        t0 = time.perf_counter()
        outs = sharded(*concat_in, *concat_zeros)
        jax.block_until_ready(outs)
        times.append(time.perf_counter() - t0)
    results = [
        {
            nm: np.asarray(outs[i]).reshape(n_cores, *out_avals[i].shape)[c]
            for i, nm in enumerate(out_names)
        }
        for c in range(n_cores)
    ]
    return times, results


def kernel(h, W, attn_l, attn_r, bias, src, dst):
    from concourse.bass_utils import run_bass_kernel_spmd

    aux, clo, chi, groups = preprocess(src, dst)
    cmaxg = max(g["C"] for g in groups)
    shared, hTs = _make_static_inputs(h, W, attn_l, attn_r, bias, cmaxg)
    nc = build_kernel(clo, chi)
    nc.compile()  # bacc passes: matmul wait splitting, event sems, DCE
    in_maps = []
    for c in range(NC):
        m = dict(shared)
        m["hT"] = hTs[c]
        m["auxw"] = aux[c]
        in_maps.append(m)
    res = run_bass_kernel_spmd(nc, in_maps, core_ids=list(range(NC)), trace=False)
    out_full = np.zeros((N, D), dtype=np.float32)
    for c in range(NC):
        dev = res.results[c]["out"]  # [TILES*P, D] head-sum; x0.25 here
        base = c * NR
        nrows = min(NR, N - base)
        out_full[base : base + nrows] = dev[:nrows] * 0.25
    kernel.last_nc = nc
    kernel.last_in_maps = in_maps
    return out_full


# revision 10
# speedup vs baseline: 27.7547x; 27.7547x over previous
"""GAT layer (AdaptiveBreadthLayer) on 8 TRN2 NeuronCores. v3.

Strategy:
  - dst-shard: core c owns destination nodes [c*6272, (c+1)*6272) (N padded
    50000 -> 50176). Every edge lives on one core (by dst); no collectives.
  - Per-core PERMUTED node order (own dst nodes first), so each core's
    member er rows sit at table rows [t*128, (t+1)*128) with uniform code.
  - Phase 1: redundant projection of all nodes. Row = 512B:
    [248 feat bf16 | 8 feat fp8 | 4 el bf16], feat (d,h)-interleaved
    (col j = (d,h), d=j//4, h=j%4) for the per-head broadcast multiply.
    Written to two DRAM halves (rows int16-indexable for dma_gather).
    Also writes er_tab [6272, 4] bf16 for the core's own dst nodes.
  - Phase 2 walks dst-tile PAIRS. Per pair: packed dma_gathers (8-chunk /
    1024-idx instructions over the pair's lo|hi chunk streams), one-hot
    (edge -> dst-slot) build on DVE, PE transposes for the er-broadcast
    matmuls, softmax without max-subtraction, fused one-hot scatter matmul
    emitting [feat 256 | denom 4], epilogue normalize + bias + tanh +
    head-sum (the x0.25 head-mean applied on host).
"""

import sys

import numpy as np

sys.path.insert(0, "/opt/trn_rl_repo")

import ml_dtypes

import concourse.bacc as bacc
import concourse.bass as bass
import concourse.mybir as mybir
from concourse.tile import TileContext

BF16 = mybir.dt.bfloat16
FP8 = mybir.dt.float8e4
F32 = mybir.dt.float32
I32 = mybir.dt.int32
I16 = mybir.dt.int16

P = 128
H = 4
D = 64
HD = H * D  # 256
ROWB = 256  # bf16-typed row width (512 bytes)
NBF = 248  # bf16 feat cols; 248:252 hold 8 fp8 feats; 252:256 el bf16
IN_DIM = 256
NEG_SLOPE = 0.2
NW = HD + 2 * H  # 264 projection cols: [feat 256 | er 4 | el 4]
GX = HD + H  # 260 scatter cols: [feat 256 | ex 4]

N = 50000
E = 800000
NC = 8
N_PAD = 50176  # 8 * 49 * 128
NR = N_PAD // NC  # 6272 dst rows per core
TILES = NR // P  # 49 dst tiles per core
HALF = N_PAD // 2  # 25088 rows per table half (int16-indexable)
PAD_DSTLOC = 200.0  # any value >= 128: matches no dst slot
MAXC = 8  # chunks per dma_gather instruction (HW ucode cap: 1024 indices)
GT = 8  # one-hot transpose group size (PSUM bank = 8*128 bf16)
G_SZ = 2  # dst tiles per gather group


# --------------------------------------------------------------------------
# group structure (shared host/device derivation)
# --------------------------------------------------------------------------

def make_groups(clo, chi):
    groups = []
    aux_off = 0
    for g0 in range(0, TILES, G_SZ):
        ts = list(range(g0, min(g0 + G_SZ, TILES)))
        clos = [int(clo[t]) for t in ts]
        chis = [int(chi[t]) for t in ts]
        CLO, CHI = sum(clos), sum(chis)
        lo_off = [0] + list(np.cumsum(clos)[:-1])
        hi_off = [0] + list(np.cumsum(chis)[:-1])
        groups.append(
            dict(
                tiles=ts,
                clos=clos,
                chis=chis,
                CLO=CLO,
                CHI=CHI,
                C=CLO + CHI,
                lo_off=lo_off,
                hi_off=hi_off,
                aux_off=aux_off,
            )
        )
        aux_off += (CLO + CHI) * 9
    return groups, aux_off


# --------------------------------------------------------------------------
# host-side preprocessing (index structures only; no float math)
# --------------------------------------------------------------------------

def preprocess(src, dst):
    src = np.asarray(src).astype(np.int64)
    dst = np.asarray(dst).astype(np.int64)
    bf = ml_dtypes.bfloat16
    core_of = dst // NR
    percore = []
    orders = []
    lo_cnt = np.zeros((NC, TILES), np.int64)
    hi_cnt = np.zeros((NC, TILES), np.int64)
    for c in range(NC):
        m = core_of == c
        s, d = src[m], dst[m]
        base = c * NR
        # in-degree balanced binning of own dst nodes into tiles (snake
        # fill by descending degree); own_order[i] = local id of the node
        # placed at permuted position i (tile i//128, slot i%128).
        indeg = np.bincount(d - base, minlength=NR)
        by_deg = np.argsort(-indeg, kind="stable")
        rounds = by_deg.reshape(P, TILES).copy()
        rounds[1::2] = rounds[1::2, ::-1]
        own_order = rounds.reshape(P, TILES).T.reshape(-1)  # pos t*P+p
        pos_of = np.empty(NR, np.int64)
        pos_of[own_order] = np.arange(NR)
        orders.append(own_order)
        # permuted position: own range (balanced order) first, then rest
        pos = np.where(
            s >= base,
            np.where(s < base + NR, -1, s),
            s + NR,
        )
        own = (s >= base) & (s < base + NR)
        pos[own] = pos_of[s[own] - base]
        is_lo = pos < HALF
        dloc = pos_of[d - base]
        tl = dloc // P
        sl = dloc % P
        percore.append((pos, is_lo, tl, sl))
        np.add.at(lo_cnt[c], tl[is_lo], 1)
        np.add.at(hi_cnt[c], tl[~is_lo], 1)
    clo = np.maximum(1, np.ceil(lo_cnt.max(axis=0) / P)).astype(np.int64)
    chi = np.maximum(1, np.ceil(hi_cnt.max(axis=0) / P)).astype(np.int64)
    clo = [int(x) for x in clo]
    chi = [int(x) for x in chi]
    groups, sum_w = make_groups(clo, chi)

    # per-tile lookup tables
    gi_of = np.zeros(TILES, np.int64)
    lo_off_t = np.zeros(TILES, np.int64)
    hi_off_t = np.zeros(TILES, np.int64)
    for gi, g in enumerate(groups):
        for j, t in enumerate(g["tiles"]):
            gi_of[t] = gi
            lo_off_t[t] = g["lo_off"][j]
            hi_off_t[t] = g["hi_off"][j]
    g_aux = np.array([g["aux_off"] for g in groups])
    g_clo = np.array([g["CLO"] for g in groups])
    g_C = np.array([g["C"] for g in groups])
    cum_C = np.concatenate([[0], np.cumsum(g_C)])
    sumC = int(cum_C[-1])

    aux = []
    for c in range(NC):
        pos, is_lo, tl, sl = percore[c]
        idxf = np.zeros((16, sumC * 8), np.int16)
        dlf = np.full((P, sumC), PAD_DSTLOC, dtype=bf)
        for half in (True, False):
            selh = is_lo == half
            t_h = tl[selh]
            p_h = sl[selh]
            pos_h = pos[selh] - (0 if half else HALF)
            order = np.argsort(t_h, kind="stable")
            t_s, pos_s, p_s = t_h[order], pos_h[order], p_h[order]
            tile_starts = np.searchsorted(t_s, np.arange(TILES))
            q = np.arange(len(order)) - tile_starts[t_s]
            chunk_local = q // P
            slot = q % P
            gi = gi_of[t_s]
            if half:
                cpos = cum_C[gi] + lo_off_t[t_s] + chunk_local
            else:
                cpos = cum_C[gi] + g_clo[gi] + hi_off_t[t_s] + chunk_local
            idxf[slot % 16, cpos * 8 + slot // 16] = pos_s.astype(np.int16)
            dlf[slot, cpos] = p_s.astype(bf)
        auxw = np.zeros((P, sum_w), np.int16)
        for gi, g in enumerate(groups):
            a0, C = g["aux_off"], g["C"]
            blk = idxf[:, cum_C[gi] * 8 : cum_C[gi + 1] * 8]
            auxw[:, a0 : a0 + C * 8] = np.tile(blk, (8, 1))
            auxw[:, a0 + C * 8 : a0 + C * 9] = dlf[
                :, cum_C[gi] : cum_C[gi + 1]
            ].view(np.int16)
        aux.append(auxw)
    return aux, clo, chi, groups, orders


# --------------------------------------------------------------------------
# device kernel builder
# --------------------------------------------------------------------------

def build_kernel(clo, chi, rep1=1, rep2=1, nq=4, scratch=32768, sp=True):
    groups, sum_w = make_groups(clo, chi)
    cmaxg = max(g["C"] for g in groups)
    nc = bacc.Bacc(dynamic_dma_scratch_size=scratch, num_swdge_queues=nq)

    hT = nc.declare_dram_parameter("hT", [IN_DIM, N_PAD], BF16, isOutput=False)
    Wfull = nc.declare_dram_parameter("Wfull", [IN_DIM, NW], BF16, isOutput=False)
    bias_rep = nc.declare_dram_parameter("bias_rep", [P, HD], F32, isOutput=False)
    iota_big = nc.declare_dram_parameter(
        "iota_big", [P, P * cmaxg], BF16, isOutput=False
    )
    ident = nc.declare_dram_parameter("ident", [P, P], BF16, isOutput=False)
    auxw = nc.declare_dram_parameter("auxw", [P, sum_w], I16, isOutput=False)
    out = nc.declare_dram_parameter("out", [TILES * P, D], F32, isOutput=True)

    AL = mybir.AluOpType
    AF = mybir.ActivationFunctionType
    KCH = IN_DIM // P  # 2 contraction chunks
    build_kernel._gq = 0

    with TileContext(nc) as tc:
        with (
            tc.tile_pool(name="const", bufs=1) as constp,
            tc.tile_pool(name="dram", bufs=1, space="DRAM") as dramp,
        ):
            t_lo = dramp.tile([HALF, ROWB], BF16)
            t_hi = dramp.tile([HALF, ROWB], BF16)
            er_tab = dramp.tile([NR, H], BF16)

            W_sb = constp.tile([P, KCH * NW], BF16)
            bias_sb = constp.tile([P, HD], F32)
            iota_sb = constp.tile([P, P * cmaxg], BF16)
            ident_sb = constp.tile([P, P], BF16)
            for kk in range(KCH):
                nc.sync.dma_start(
                    out=W_sb[:, kk * NW : (kk + 1) * NW],
                    in_=Wfull[kk * P : (kk + 1) * P, :],
                )
            nc.sync.dma_start(out=bias_sb[:], in_=bias_rep[:, :])
            nc.sync.dma_start(out=iota_sb[:], in_=iota_big[:, :])
            nc.sync.dma_start(out=ident_sb[:], in_=ident[:, :])
            aux_all = constp.tile([P, sum_w], I16)
            nc.sync.dma_start(out=aux_all[:], in_=auxw[:, :])
            out_all = constp.tile([P, TILES, D], F32)
            er_all = constp.tile([P, TILES, H], BF16)

            # ------------------- phase 1: projection table -------------------
            OB = 512  # rows per outer block
            SUBS = OB // P  # 4
            n_ob = N_PAD // OB
            SPL = 152  # stage copy split: Act does 0:SPL, DVE does SPL:NBF
            with (
                tc.tile_pool(name="p1", bufs=3) as p1,
                tc.tile_pool(name="p1ps", bufs=2, space="PSUM") as p1ps,
            ):
              for _r1 in range(rep1):
                hT_tiles = {}

                def load_block(tt):
                    ht = p1.tile([P, KCH, OB], BF16, name="hT_t", tag="hT_t")
                    nc.sync.dma_start(
                        out=ht[:],
                        in_=hT[:, tt * OB : (tt + 1) * OB].rearrange(
                            "(k p) c -> p k c", p=P
                        ),
                    )
                    hT_tiles[tt] = ht

                load_block(0)
                load_block(1)
                for ob in range(n_ob):
                    start = ob * OB
                    hT_t = hT_tiles.pop(ob)
                    # 4 banks: sub s occupies bank s: [feat 256 | er 4 | el 4]
                    pbig = p1ps.tile([P, SUBS, 512], F32, name="pbig", tag="pbig")
                    for sub in range(SUBS):
                        for kk in range(KCH):
                            nc.tensor.matmul(
                                pbig[:, sub, 0:NW],
                                lhsT=hT_t[:, kk, sub * P : (sub + 1) * P],
                                rhs=W_sb[:, kk * NW : (kk + 1) * NW],
                                start=(kk == 0),
                                stop=(kk == KCH - 1),
                            )
                    stage = p1.tile([P, SUBS, ROWB], BF16, name="stage", tag="stage")
                    nc.scalar.copy(out=stage[:, :, 0:SPL], in_=pbig[:, :, 0:SPL])
                    nc.vector.tensor_copy(
                        out=stage[:, :, SPL:NBF], in_=pbig[:, :, SPL:NBF]
                    )
                    # 8 fp8 feats <- psum cols 248:256
                    nc.vector.tensor_copy(
                        out=stage[:, :, NBF : NBF + 4].bitcast(FP8),
                        in_=pbig[:, :, NBF : NBF + 8],
                    )
                    # el bf16 <- psum cols 260:264
                    nc.vector.tensor_copy(
                        out=stage[:, :, 252:256], in_=pbig[:, :, 260:264]
                    )
                    if start < NR:
                        nsub = SUBS if start + OB <= NR else (NR - start) // P
                        er_st = p1.tile([P, SUBS, H], BF16, name="er_st", tag="er_st")
                        nc.vector.tensor_copy(
                            out=er_st[:, 0:nsub, :], in_=pbig[:, 0:nsub, 256:260]
                        )
                        nc.sync.dma_start(
                            out=er_tab[start : start + nsub * P, :].rearrange(
                                "(s p) h -> p s h", p=P
                            ),
                            in_=er_st[:, 0:nsub, :],
                        )
                    # prefetch next-next hT block behind the stage copies
                    if ob + 2 < n_ob:
                        load_block(ob + 2)
                    tgt, r0 = (t_lo, start) if start < HALF else (t_hi, start - HALF)
                    dst_ap = tgt[r0 : r0 + OB, :].rearrange("(s p) c -> p s c", p=P)
                    nc.sync.dma_start(out=dst_ap, in_=stage[:])

            # ------------------- phase 2: edge aggregation -------------------
            with (
                tc.tile_pool(name="p2", bufs=3) as p2,
                tc.tile_pool(name="p2g", bufs=2) as p2g,
                tc.tile_pool(name="p2s", bufs=4) as p2s,
                tc.tile_pool(name="ssb", bufs=6) as ssb_pool,
                tc.tile_pool(name="outps", bufs=4, space="PSUM") as outps_pool,
                tc.tile_pool(name="ergps", bufs=2, space="PSUM") as ergps_pool,
                tc.tile_pool(name="sps", bufs=2, space="PSUM") as sps_pool,
            ):
              for _r2 in range(rep2):
                nc.sync.dma_start(
                    out=er_all[:],
                    in_=er_tab[:, :].rearrange("(t p) h -> p t h", p=P),
                )
                for g in groups:
                    C, CLO, CHI = g["C"], g["CLO"], g["CHI"]
                    a0 = g["aux_off"]
                    aux_g = aux_all[:, a0 : a0 + C * 9]
                    idx_v = aux_g[:, 0 : C * 8]
                    dl_v = aux_g[:, C * 8 : C * 9].bitcast(BF16)

                    G = p2g.tile([P, C, ROWB], BF16, name="G", tag="G")
                    for b0, width, tb in ((0, CLO, t_lo), (CLO, CHI, t_hi)):
                        done = 0
                        while done < width:
                            w = min(MAXC, width - done)
                            b = b0 + done
                            nc.gpsimd.dma_gather(
                                out_ap=G[:, b : b + w, :],
                                in_ap=tb[:, :],
                                idxs_ap=idx_v[:, b * 8 : (b + w) * 8],
                                num_idxs=w * P,
                                num_idxs_reg=w * P,
                                elem_size=ROWB,
                                queue_num=build_kernel._gq % nq,
                                single_packet=sp,
                            )
                            build_kernel._gq += 1
                            done += w

                    # one-hot over all the group's chunks, chunk-innermost
                    # (DVE 2x): ST3[e, d, j] = (dstloc[e, j] == d)
                    ST3 = p2.tile([P, P * C], BF16, name="ST3", tag="ST")
                    nc.vector.tensor_tensor(
                        out=ST3[:].rearrange("p (d c) -> p d c", d=P),
                        in0=dl_v.rearrange("p (one c) -> p one c", one=1)
                        .to_broadcast([P, P, C]),
                        in1=iota_sb[:]
                        .rearrange("p (d c) -> p d c", d=P)[:, :, 0:C],
                        op=AL.is_equal,
                    )
                    st_j = lambda j: ST3[:].rearrange("p (d c) -> p d c", d=P)[
                        :, :, j
                    ]

                    # transposed one-hot blocks (for the erg matmuls)
                    sbs = []
                    for q0 in range(0, C, GT):
                        q1 = min(q0 + GT, C)
                        s_ps = sps_pool.tile(
                            [P, GT, P], BF16, name="s_ps", tag="s_ps"
                        )
                        for j in range(q0, q1):
                            nc.tensor.transpose(
                                out=s_ps[:, j - q0, :],
                                in_=st_j(j),
                                identity=ident_sb[:],
                            )
                        s_sb = ssb_pool.tile(
                            [P, GT, P], BF16, name="s_sb", tag="s_sb"
                        )
                        nc.scalar.copy(
                            out=s_sb[:, 0 : q1 - q0, :], in_=s_ps[:, 0 : q1 - q0, :]
                        )
                        sbs.append((q0, q1, s_sb))

                    # chunk -> tile map (group chunk order: los then his)
                    t_of = []
                    for j_t, t in enumerate(g["tiles"]):
                        t_of += [t] * g["clos"][j_t]
                    for j_t, t in enumerate(g["tiles"]):
                        t_of += [t] * g["chis"][j_t]

                    # er gathered per edge: erg[e, h] = er_all[dstloc[e], t, h]
                    erg_ps = ergps_pool.tile([P, C * H], F32, name="erg_ps")
                    for q0, q1, s_sb in sbs:
                        for j in range(q0, q1):
                            nc.tensor.matmul(
                                erg_ps[:, j * H : (j + 1) * H],
                                lhsT=s_sb[:, j - q0, :],
                                rhs=er_all[:, t_of[j], :],
                                start=True,
                                stop=True,
                            )

                    # e_val = leaky_relu(el[src] + er[dst]); ex = exp(e_val)
                    ev = p2.tile([P, C * H], F32, name="ev", tag="ev")
                    nc.vector.tensor_tensor(
                        out=ev[:].rearrange("p (c h) -> p c h", c=C),
                        in0=G[:, :, 252:256],
                        in1=erg_ps[:].rearrange("p (c h) -> p c h", c=C),
                        op=AL.add,
                    )
                    lrel = p2.tile([P, C * H], F32, name="lrel", tag="lrel")
                    nc.vector.scalar_tensor_tensor(
                        out=lrel[:],
                        in0=ev[:],
                        scalar=NEG_SLOPE,
                        in1=ev[:],
                        op0=AL.mult,
                        op1=AL.max,
                    )
                    gx = p2.tile([P, C, GX], BF16, name="gx", tag="gx")
                    nc.scalar.activation(
                        out=gx[:, :, HD:GX],
                        in_=lrel[:].rearrange("p (c h) -> p c h", c=C),
                        func=AF.Exp,
                    )
                    exv = gx[:, :, HD:GX].rearrange(
                        "p c (one h) -> p c one h", one=1
                    )
                    # 8 fp8 feats -> bf16
                    Gx8 = p2.tile([P, C, 8], BF16, name="Gx8", tag="Gx8")
                    nc.scalar.copy(
                        out=Gx8[:], in_=G[:, :, NBF : NBF + 4].bitcast(FP8)
                    )
                    nc.vector.tensor_tensor(
                        out=gx[:, :, 0:NBF].rearrange("p c (d h) -> p c d h", h=H),
                        in0=G[:, :, 0:NBF].rearrange("p c (d h) -> p c d h", h=H),
                        in1=exv.to_broadcast([P, C, NBF // H, H]),
                        op=AL.mult,
                    )
                    nc.vector.tensor_tensor(
                        out=gx[:, :, NBF:HD].rearrange("p c (d h) -> p c d h", h=H),
                        in0=Gx8[:].rearrange("p c (d h) -> p c d h", h=H),
                        in1=exv.to_broadcast([P, C, (HD - NBF) // H, H]),
                        op=AL.mult,
                    )

                    # scatter matmuls + epilogue, one dst tile at a time
                    for j_t, t in enumerate(g["tiles"]):
                        chunks = list(
                            range(g["lo_off"][j_t], g["lo_off"][j_t] + g["clos"][j_t])
                        ) + list(
                            range(
                                CLO + g["hi_off"][j_t],
                                CLO + g["hi_off"][j_t] + g["chis"][j_t],
                            )
                        )
                        out_ps = outps_pool.tile([P, GX], F32, name="out_ps")
                        for k, j in enumerate(chunks):
                            nc.tensor.matmul(
                                out_ps[:],
                                lhsT=st_j(j),
                                rhs=gx[:, j, :],
                                start=(k == 0),
                                stop=(k == len(chunks) - 1),
                            )
                        # normalize, bias, tanh, head-sum (x0.25 on host)
                        rd = p2s.tile([P, H], F32, name="rd", tag="rd")
                        nc.vector.reciprocal(out=rd[:], in_=out_ps[:, HD:GX])
                        nrm = p2.tile([P, HD], F32, name="nrm", tag="nrm")
                        nc.vector.tensor_tensor(
                            out=nrm[:].rearrange("p (d h) -> p d h", h=H),
                            in0=out_ps[:, 0:HD].rearrange("p (d h) -> p d h", h=H),
                            in1=rd[:]
                            .rearrange("p (one h) -> p one h", one=1)
                            .to_broadcast([P, D, H]),
                            op=AL.mult,
                        )
                        nb = p2.tile([P, HD], F32, name="nb", tag="nb")
                        nc.vector.tensor_tensor(
                            out=nb[:], in0=nrm[:], in1=bias_sb[:], op=AL.add
                        )
                        th = p2.tile([P, HD], F32, name="th", tag="th")
                        nc.scalar.activation(out=th[:], in_=nb[:], func=AF.Tanh)
                        nc.vector.tensor_reduce(
                            out=out_all[:, t, :],
                            in_=th[:].rearrange("p (d h) -> p d h", h=H),
                            axis=mybir.AxisListType.X,
                            op=AL.add,
                        )
                nc.sync.dma_start(
                    out=out[:, :].rearrange("(t p) d -> p t d", p=P), in_=out_all[:]
                )
    return nc


# --------------------------------------------------------------------------
# host entry
# --------------------------------------------------------------------------

# feat col j = (d, h): feat_new[:, j] = feat_old[:, (j % 4) * 64 + j // 4]
_PERM = np.array([(j % H) * D + j // H for j in range(HD)], dtype=np.int64)


def _make_static_inputs(h, W, attn_l, attn_r, bias, cmaxg, orders):
    bf = ml_dtypes.bfloat16
    h_pad = np.zeros((N_PAD, IN_DIM), dtype=np.float32)
    h_pad[:N] = np.asarray(h, dtype=np.float32)
    W_perm = np.asarray(W, dtype=np.float32)[:, _PERM]
    ALRm = np.zeros((HD, 2 * H), dtype=np.float32)
    al = np.asarray(attn_l, dtype=np.float32)
    ar = np.asarray(attn_r, dtype=np.float32)
    for j in range(HD):
        d, hh = j // H, j % H
        ALRm[j, hh] = ar[hh, d]  # er first
        ALRm[j, H + hh] = al[hh, d]  # el last
    Wfull = np.concatenate([W_perm, W_perm @ ALRm], axis=1).astype(bf)
    bias_perm = np.asarray(bias, dtype=np.float32)[_PERM]
    bias_rep = np.tile(bias_perm.reshape(1, HD), (P, 1))
    ident = np.eye(P, dtype=np.float32).astype(bf)
    # iota_big[p, d*cmaxg + j] = d  (chunk-innermost iota for the ST3 build)
    row = np.repeat(np.arange(P, dtype=np.float32), cmaxg).reshape(1, P * cmaxg)
    iota_big = np.ascontiguousarray(np.tile(row, (P, 1)).astype(bf))
    shared = dict(
        Wfull=np.ascontiguousarray(Wfull),
        bias_rep=np.ascontiguousarray(bias_rep),
        ident=np.ascontiguousarray(ident),
        iota_big=iota_big,
    )
    hTs = []
    for c in range(NC):
        base = c * NR
        order = np.concatenate(
            [
                base + orders[c],
                np.arange(0, base),
                np.arange(base + NR, N_PAD),
            ]
        )
        hTs.append(np.ascontiguousarray(h_pad[order].T.astype(bf)))
    return shared, hTs


def bench(nc, in_maps, n_iters=10):
    """Repeated-execution wall timing of the compiled SPMD kernel via PJRT.

    Returns (per_call_seconds_list, results_of_last_call)."""
    import time

    import jax
    from jax.sharding import Mesh, NamedSharding, PartitionSpec
    from jax.experimental.shard_map import shard_map

    from concourse import bass2jax, mybir as _mb

    bass2jax.install_neuronx_cc_hook()
    n_cores = len(in_maps)
    in_names, out_names, out_avals, zero_outs = [], [], [], []
    partition_name = nc.partition_id_tensor.name if nc.partition_id_tensor else None
    for alloc in nc.m.functions[0].allocations:
        if not isinstance(alloc, _mb.MemoryLocationSet):
            continue
        name = alloc.memorylocations[0].name
        if alloc.kind == "ExternalInput":
            if name != partition_name:
                in_names.append(name)
        elif alloc.kind == "ExternalOutput":
            out_names.append(name)
            shape = tuple(alloc.tensor_shape)
            dtype = _mb.dt.np(alloc.dtype)
            out_avals.append(jax.core.ShapedArray(shape, dtype))
            zero_outs.append(np.zeros(shape, dtype))
    n_params = len(in_names)
    all_in_names = in_names + out_names
    if partition_name is not None:
        all_in_names.append(partition_name)

    def _body(*args):
        operands = list(args)
        if partition_name is not None:
            operands.append(bass2jax.partition_id_tensor())
        outs = bass2jax._bass_exec_p.bind(
            *operands,
            out_avals=tuple(out_avals),
            in_names=tuple(all_in_names),
            out_names=tuple(out_names),
            lowering_input_output_aliases=(),
            sim_require_finite=True,
            sim_require_nnan=True,
            nc=nc,
        )
        return tuple(outs)

    devices = jax.devices()[:n_cores]
    mesh = Mesh(np.asarray(devices), ("core",))
    n_outs = len(out_names)
    sharded = jax.jit(
        shard_map(
            _body,
            mesh=mesh,
            in_specs=(PartitionSpec("core"),) * (n_params + n_outs),
            out_specs=(PartitionSpec("core"),) * n_outs,
            check_rep=False,
        ),
        keep_unused=True,
    )
    sh = NamedSharding(mesh, PartitionSpec("core"))
    concat_in = [
        jax.device_put(
            np.concatenate([np.asarray(in_maps[c][nm]) for c in range(n_cores)], 0), sh
        )
        for nm in in_names
    ]
    concat_zeros = [
        jax.device_put(np.zeros((n_cores * z.shape[0], *z.shape[1:]), z.dtype), sh)
        for z in zero_outs
    ]
    outs = sharded(*concat_in, *concat_zeros)  # warmup/compile
    jax.block_until_ready(outs)
    times = []
    for _ in range(n_iters):
        t0 = time.perf_counter()
        outs = sharded(*concat_in, *concat_zeros)
        jax.block_until_ready(outs)
        times.append(time.perf_counter() - t0)
    results = [
        {
            nm: np.asarray(outs[i]).reshape(n_cores, *out_avals[i].shape)[c]
            for i, nm in enumerate(out_names)
        }
        for c in range(n_cores)
    ]
    return times, results


def kernel(h, W, attn_l, attn_r, bias, src, dst):
    from concourse.bass_utils import run_bass_kernel_spmd

    aux, clo, chi, groups, orders = preprocess(src, dst)
    cmaxg = max(g["C"] for g in groups)
    shared, hTs = _make_static_inputs(h, W, attn_l, attn_r, bias, cmaxg, orders)
    nc = build_kernel(clo, chi)
    nc.compile()  # bacc passes: matmul wait splitting, event sems, DCE
    in_maps = []
    for c in range(NC):
        m = dict(shared)
        m["hT"] = hTs[c]
        m["auxw"] = aux[c]
        in_maps.append(m)
    res = run_bass_kernel_spmd(nc, in_maps, core_ids=list(range(NC)), trace=False)
    out_full = np.zeros((N, D), dtype=np.float32)
    for c in range(NC):
        dev = res.results[c]["out"]  # [TILES*P, D] head-sum; x0.25 here
        base = c * NR
        rows = base + orders[c]  # permuted position i holds node rows[i]
        valid = rows < N
        out_full[rows[valid]] = dev[valid] * 0.25
    kernel.last_nc = nc
    kernel.last_in_maps = in_maps
    return out_full
